# revision 12
# baseline (speedup 1.0000x reference)
"""Trainium2 Bass kernel for the gated-MLP-over-ring-buffer problem.

Reference computation (B=512, M=128, V=256, H=256, IN = M*V = 32768):
    mem    = roll(memory, 1, axis=1); mem[:, 0, :] = x        # [B, M, V]
    flat   = mem.reshape(B, IN)                                # [B, 32768]
    h      = tanh(flat @ W1 + b1) * sigmoid(flat @ Wg + bg)    # [B, 256]
    logits = h @ W2 + b2                                       # [B, 256]

Strategy (8 NeuronCores, one trn2 chip):
  - Contraction-shard the two big GEMMs: core c owns k-rows
    [4096c, 4096(c+1)) of W1/Wg and the matching slab of flat.T
    (host-prepared, transposed so SBUF tiles load at line rate).
  - Each core computes partial P1.T / Pg.T = W.T @ flat.T  -> [H, B]
    accumulated over its 32 k-chunks in PSUM (bf16 operands, f32
    accumulate by default; float32r variant available via env).
  - Cross-core reduction of the [2H, B] partials, scattered over B so
    core c ends up with batch columns [64c, 64c+64).
  - Each core applies bias + tanh/sigmoid gating and the small W2
    GEMM for its batch chunk, writing logits.T [V, 64].
  - Host assembles/transposes the 8 chunks back to [B, V].
"""

import os

import numpy as np

import concourse.bacc as bacc
import concourse.bass as bass
import concourse.mybir as mybir
import concourse.tile as tile
from concourse import bass_utils

B, M, V, H = 512, 128, 256, 256
IN = M * V              # 32768
NCORES = 8
KC = IN // NCORES       # 4096 contraction rows per core
NKG = 8                 # DMA k-groups per core
KB_PER_G = KC // (NKG * 128)  # 4 k-chunks of 128 per group
BCHUNK = B // NCORES    # 64 batch columns per core after reduce-scatter

F32 = mybir.dt.float32
F32R = mybir.dt.float32r
BF16 = mybir.dt.bfloat16
AF = mybir.ActivationFunctionType

VARIANT = os.environ.get("KERNEL_VARIANT", "p2b")

_CACHE = {}


def _stage2(nc, s2pool, psum2, s2, bt, w2t, outT):
    """Gate + W2 for the local batch chunk.

    s2: SBUF AP [128, 4, BCHUNK] holding [p1_h0, p1_h1, pg_h0, pg_h1]
    row-blocks of the fully-reduced partials for this core's chunk.
    """
    hT = []
    for i in range(2):
        th = s2pool.tile([128, BCHUNK], F32, tag=f"th{i}", name=f"th{i}")
        nc.scalar.activation(th[:], s2[:, i, :], AF.Tanh, bias=bt[:, i : i + 1])
        sg = s2pool.tile([128, BCHUNK], F32, tag=f"sg{i}", name=f"sg{i}")
        nc.scalar.activation(
            sg[:], s2[:, 2 + i, :], AF.Sigmoid, bias=bt[:, 2 + i : 3 + i]
        )
        ht = s2pool.tile([128, BCHUNK], F32R, tag=f"ht{i}", name=f"ht{i}")
        nc.vector.tensor_mul(ht[:], th[:].bitcast(F32R), sg[:].bitcast(F32R))
        hT.append(ht)

    for v in range(2):
        ps = psum2.tile([128, BCHUNK], F32, tag=f"acc{v}", name=f"ps2_{v}")
        for i in range(2):
            nc.tensor.matmul(
                ps[:],
                w2t[:, i, bass.ts(v, 128)],
                hT[i][:],
                start=(i == 0),
                stop=(i == 1),
            )
        ot = s2pool.tile([128, BCHUNK], F32, tag=f"ot{v}", name=f"ot{v}")
        nc.vector.tensor_scalar_add(ot[:], ps[:], bt[:, 4 + v : 5 + v])
        nc.sync.dma_start(out=outT[bass.ts(v, 128), :], in_=ot[:])


def _build(variant=VARIANT):
    nc = bacc.Bacc(
        "TRN2",
        target_bir_lowering=False,
        debug=False,
        enable_asserts=False,
        num_devices=NCORES,
    )

    bf16_in = variant.endswith("b")
    DT_IN = BF16 if bf16_in else F32
    DT_MM = BF16 if bf16_in else F32R

    # Per-core external inputs.
    memT = nc.dram_tensor("memT", [NKG, KB_PER_G, 128, B], DT_IN, kind="ExternalInput")
    w1 = nc.dram_tensor("w1", [NKG, KB_PER_G, 128, H], DT_IN, kind="ExternalInput")
    wg = nc.dram_tensor("wg", [NKG, KB_PER_G, 128, H], DT_IN, kind="ExternalInput")
    w2 = nc.dram_tensor("w2", [H, V], F32, kind="ExternalInput")
    # packed biases: cols = [b1_lo, b1_hi, bg_lo, bg_hi, b2_lo, b2_hi]
    bpk = nc.dram_tensor("bpk", [128, 6], F32, kind="ExternalInput")
    outT = nc.dram_tensor("outT", [V, BCHUNK], F32, kind="ExternalOutput")

    def mm_in(ap):
        return ap if bf16_in else ap.bitcast(F32R)

    with tile.TileContext(nc) as tc:
        with (
            tc.tile_pool(name="xg", bufs=3) as xpool,
            tc.tile_pool(name="wt", bufs=3) as wpool,
            tc.tile_pool(name="part", bufs=1) as ppool,
            tc.tile_pool(name="s2", bufs=1) as s2pool,
            tc.tile_pool(name="psum1", bufs=1, space="PSUM") as psum1,
            tc.tile_pool(name="dram", bufs=1, space="DRAM") as dpool,
        ):
            # Pre-warm the Tanh/Sigmoid activation tables off the critical
            # path (the first use of each table pays a ~1.3us load).
            warm = s2pool.tile([128, 1], F32, tag="warm")
            nc.gpsimd.memset(warm[:], 0.0)
            warm2 = s2pool.tile([128, 1], F32, tag="warm2")
            nc.scalar.activation(warm2[:], warm[:], AF.Tanh)
            nc.scalar.activation(warm[:], warm2[:], AF.Sigmoid)

            # Pre-warm the PE HAM clock gate with dummy matmuls while the
            # first input DMAs are in flight (~4us of sustained PE activity
            # releases the 4/8 throttle).
            wsrc = s2pool.tile([128, B], BF16, tag="wsrc")
            nc.gpsimd.memset(wsrc[:], 0.0)
            wps = psum1.tile([128, B], F32, tag="acc7", name="wps")
            for i in range(20):
                nc.tensor.matmul(
                    wps[:], wsrc[:, 0:128], wsrc[:], start=(i == 0), stop=(i == 19)
                )

            # Stage-2 constants on the (otherwise idle) gpsimd SWDGE queue.
            bt = s2pool.tile([128, 6], F32, tag="bias")
            nc.gpsimd.dma_start(out=bt[:], in_=bpk[:, :])
            w2t = s2pool.tile([128, 2, V], F32R, tag="w2")
            nc.gpsimd.dma_start(
                out=w2t[:], in_=w2.rearrange("(c p) v -> p c v", p=128).bitcast(F32R)
            )

            # ---------------- stage 1: partial W.T @ flat.T ----------------
            # k-split halves: acc_h[0] accumulates k-groups 0..3, acc_h[1]
            # k-groups 4..7; each half reduces across cores independently so
            # the first collective hides under the second half of compute.
            acc = [
                [
                    psum1.tile([128, B], F32, tag=f"acc{4 * s + i}", name=f"acc{s}_{i}")
                    for i in range(4)
                ]
                for s in range(2)
            ]

            ccin = [None, None]
            ccout = [None, None]
            for s in range(2):
                ccin[s] = dpool.tile(
                    [NCORES, 4 * 128, BCHUNK], F32, tag=f"ccin{s}", name=f"ccin{s}"
                )
                ccout[s] = dpool.tile(
                    [NCORES, 4 * 128, BCHUNK], F32, tag=f"ccout{s}", name=f"ccout{s}"
                )

            def flush_half(s):
                # PSUM -> SBUF -> DRAM scatter layout -> AllToAll
                for t in range(4):
                    sbt = ppool.tile([128, B], F32, tag=f"po{t}", name=f"po{s}_{t}")
                    nc.vector.tensor_copy(sbt[:], acc[s][t][:])
                    nc.gpsimd.dma_start(
                        out=ccin[s][:, bass.ts(t, 128), :].rearrange("c p b -> p c b"),
                        in_=sbt[:].rearrange("p (c b) -> p c b", c=NCORES),
                    )
                nc.gpsimd.collective_compute(
                    "AllToAll",
                    mybir.AluOpType.bypass,
                    replica_groups=[list(range(NCORES))],
                    ins=[ccin[s][:].opt()],
                    outs=[ccout[s][:].opt()],
                )

            SPLIT_G = 3  # k-groups in the first (early-flushed) half
            for kg in range(NKG):
                s = 0 if kg < SPLIT_G else 1
                xg = xpool.tile([128, KB_PER_G, B], DT_MM, tag="xg")
                nc.sync.dma_start(
                    out=xg[:], in_=mm_in(memT[kg].rearrange("g p b -> p g b"))
                )
                w1t = wpool.tile([128, KB_PER_G, H], DT_MM, tag="w1t")
                nc.scalar.dma_start(
                    out=w1t[:], in_=mm_in(w1[kg].rearrange("g p h -> p g h"))
                )
                wgt = wpool.tile([128, KB_PER_G, H], DT_MM, tag="wgt")
                nc.scalar.dma_start(
                    out=wgt[:], in_=mm_in(wg[kg].rearrange("g p h -> p g h"))
                )
                for kb in range(KB_PER_G):
                    k = kg * KB_PER_G + kb
                    lo = 0 if s == 0 else SPLIT_G * KB_PER_G
                    hi = SPLIT_G * KB_PER_G if s == 0 else NKG * KB_PER_G
                    first = k == lo
                    last = k == hi - 1
                    rhs = xg[:, kb, :]
                    for h in range(2):
                        nc.tensor.matmul(
                            acc[s][h][:],
                            w1t[:, kb, bass.ts(h, 128)],
                            rhs,
                            start=first,
                            stop=last,
                        )
                        nc.tensor.matmul(
                            acc[s][2 + h][:],
                            wgt[:, kb, bass.ts(h, 128)],
                            rhs,
                            start=first,
                            stop=last,
                        )
                if kg == SPLIT_G - 1:
                    flush_half(0)
            flush_half(1)

            # ------------- local reduction of received slabs -------------
            # half 0 folds completely while collective #2 is in flight;
            # after rr1 lands only 3 serial DVE ops remain.
            rf = [None, None]
            for s in range(2):
                rr = s2pool.tile(
                    [128, NCORES, 4, BCHUNK], F32, tag=f"rr{s}", name=f"rr{s}"
                )
                nc.gpsimd.dma_start(
                    out=rr[:],
                    in_=ccout[s].rearrange("c (t p) b -> p c t b", p=128),
                )
                t1 = s2pool.tile(
                    [128, 4, 4, BCHUNK], F32, tag=f"t1{s}", name=f"t1{s}"
                )
                nc.vector.tensor_add(t1[:], rr[:, 0:4, :, :], rr[:, 4:8, :, :])
                t2 = s2pool.tile(
                    [128, 2, 4, BCHUNK], F32, tag=f"t2{s}", name=f"t2{s}"
                )
                nc.vector.tensor_add(t2[:], t1[:, 0:2, :, :], t1[:, 2:4, :, :])
                rf[s] = s2pool.tile(
                    [128, 4, BCHUNK], F32, tag=f"rf{s}", name=f"rf{s}"
                )
                nc.vector.tensor_add(rf[s][:], t2[:, 0, :, :], t2[:, 1, :, :])
            s2 = s2pool.tile([128, 4, BCHUNK], F32, tag="s2in")
            nc.vector.tensor_add(s2[:], rf[0][:], rf[1][:])

            # ---------------- stage 2: gate + W2 ----------------
            _stage2(nc, s2pool, psum1, s2, bt, w2t, outT)

    nc.compile()
    return nc


def _shard(x, memory, W1, b1, Wg, bg, W2, b2, variant=VARIANT):
    """Build the 8 per-core input maps from the full problem inputs."""
    import ml_dtypes

    dt_in = ml_dtypes.bfloat16 if variant.endswith("b") else np.float32
    x = np.asarray(x, dtype=np.float32)
    memory = np.asarray(memory, dtype=np.float32)
    W1 = np.asarray(W1, dtype=np.float32)
    Wg = np.asarray(Wg, dtype=np.float32)
    W2 = np.ascontiguousarray(np.asarray(W2, dtype=np.float32))
    b1 = np.asarray(b1, dtype=np.float32)
    bg = np.asarray(bg, dtype=np.float32)
    b2 = np.asarray(b2, dtype=np.float32)

    # rolled ring buffer, flattened and transposed: [IN, B]
    flatT = np.empty((IN, B), dtype=np.float32)
    flatT[:V] = x.T
    flatT[V:] = memory[:, : M - 1, :].reshape(B, IN - V).T
    bpk = np.ascontiguousarray(
        np.stack([b1[:128], b1[128:], bg[:128], bg[128:], b2[:128], b2[128:]], axis=1)
    )

    in_maps = []
    for c in range(NCORES):
        sl = slice(KC * c, KC * (c + 1))
        in_maps.append(
            {
                "memT": flatT[sl].astype(dt_in).reshape(NKG, KB_PER_G, 128, B),
                "w1": W1[sl].astype(dt_in).reshape(NKG, KB_PER_G, 128, H),
                "wg": Wg[sl].astype(dt_in).reshape(NKG, KB_PER_G, 128, H),
                "w2": W2,
                "bpk": bpk,
            }
        )
    return in_maps


def _get_nc():
    if "nc" not in _CACHE:
        _CACHE["nc"] = _build()
    return _CACHE["nc"]


def kernel(x, memory, W1, b1, Wg, bg, W2, b2, **run_kwargs):
    nc = _get_nc()
    in_maps = _shard(x, memory, W1, b1, Wg, bg, W2, b2)
    res = bass_utils.run_bass_kernel_spmd(
        nc, in_maps, core_ids=list(range(NCORES)), **run_kwargs
    )
    _CACHE["last_results"] = res
    out = np.empty((B, V), dtype=np.float32)
    for c in range(NCORES):
        out[c * BCHUNK : (c + 1) * BCHUNK, :] = res.results[c]["outT"].T
    return out
